# revision 1
# baseline (speedup 1.0000x reference)
"""Trainium2 Bass kernel for nn_Deconv (NMF-style deconvolution, B=8, C=64, SRC=16, 256x256, K=5).

Sharding: pure data-parallel over batch. Each of the 8 NeuronCores processes one
sample end-to-end; h0/W_lin/b_lin are replicated. All convolutions run as
PSUM-accumulated f32r matmuls in channel-major layout; the full-image
correlations (sconv) run as pixel-contraction matmuls over PE-transposed row
tiles, accumulated into a single PSUM bank across the whole image.
"""

import sys

sys.path.insert(0, "/opt/trn_rl_repo")

import numpy as np

import concourse.bass as bass  # noqa: F401
import concourse.tile as tile
from concourse import bacc, mybir

F32 = mybir.dt.float32
F32R = mybir.dt.float32r

B, C, S = 8, 64, 16
H = W = 256
KK = 5
PADW = 260
EPS = 1e-16
N_CORES = 8

_CACHE = {}


def _emit(nc, tc, x_in, h0_in, wlt_in, bl_in, id_in, zr_in, out_d):
    from contextlib import ExitStack

    ctx = ExitStack()
    with ctx:
        dram = ctx.enter_context(tc.tile_pool(name="dram", bufs=1, space="DRAM"))
        consts = ctx.enter_context(tc.tile_pool(name="consts", bufs=1))

        # ---- DRAM intermediates ----
        x_pad = dram.tile([C, PADW, PADW], F32R, tag="x_pad", name="x_pad")
        s_a = dram.tile([S, PADW, PADW], F32R, tag="s_a", name="s_a")
        s_b = dram.tile([S, PADW, PADW], F32R, tag="s_b", name="s_b")
        xh_a = dram.tile([C, PADW, PADW], F32R, tag="xh_a", name="xh_a")
        xh_b = dram.tile([C, PADW, PADW], F32R, tag="xh_b", name="xh_b")
        num_t = dram.tile([S, H, W], F32, tag="num_t", name="num_t")
        den_t = dram.tile([S, H, W], F32, tag="den_t", name="den_t")

        # ---- consts ----
        identr = consts.tile([128, 128], F32R, tag="identr", name="identr")
        nc.sync.dma_start(identr[:], id_in[:])
        wlt = consts.tile([C, S], F32R, tag="wlt", name="wlt")
        nc.sync.dma_start(wlt[:], wlt_in[:])
        blt = consts.tile([S, 1], F32, tag="blt", name="blt")
        nc.sync.dma_start(blt[:], bl_in[:])
        epst = consts.tile([128, 1], F32, tag="epst", name="epst")
        nc.gpsimd.memset(epst[:], EPS)
        zt = consts.tile([128, 520], F32R, tag="zt", name="zt")
        nc.sync.dma_start(zt[:], zr_in[:])

        hv = [consts.tile([C, S * 25], F32R, tag=f"h_v{v}", name=f"h_v{v}") for v in range(3)]
        h2v = [consts.tile([128, S * 25], F32R, tag=f"h2_v{v}", name=f"h2_v{v}") for v in range(3)]
        hT5v = [consts.tile([S * KK, KK, C], F32R, tag=f"hT5_v{v}", name=f"hT5_v{v}") for v in range(3)]
        hBv = [consts.tile([C, S * 25], F32R, tag=f"hB_v{v}", name=f"hB_v{v}") for v in range(3)]
        nc.sync.dma_start(hv[0][:], h0_in[:])

        # ---- zero borders of padded tensors ----
        for t, ch in ((x_pad, C), (s_a, S), (s_b, S), (xh_a, C), (xh_b, C)):
            nc.sync.dma_start(t[:, 0:2, :], zt[0:ch, 0:520])
            nc.sync.dma_start(t[:, 258:260, :], zt[0:ch, 0:520])
            for rh in range(4):
                r0 = 2 + 64 * rh
                nc.sync.dma_start(t[:, r0:r0 + 64, 0:2], zt[0:ch, 0:128])
                nc.sync.dma_start(t[:, r0:r0 + 64, 258:260], zt[0:ch, 0:128])

        def build_weights(v):
            """From hv[v] build h2v[v] (plain + col-shift-by-1 halves) and hT5v[v]."""
            h = hv[v]
            h2 = h2v[v]
            nc.vector.tensor_copy(h2[0:C, :], h[:, :])
            nc.vector.tensor_copy(h2[C:128, 1:S * 25], h[:, 0:S * 25 - 1])
            # hB[c, (dy, g, i)] = h[c, (i, dy, g)] — permuted copy so that each
            # dy-slice is one contiguous 80-column run in (g, i) order.
            hB = hBv[v]
            hbo = hB[:].rearrange("c (dy g i) -> c dy g i", dy=KK, g=KK, i=S)
            hbi = (
                h[:]
                .rearrange("c (i dy g) -> c i dy g", i=S, dy=KK, g=KK)
                .rearrange("c i dy g -> c dy g i")
            )
            nc.vector.tensor_copy(hbo, hbi)
            with tc.tile_pool(name=f"wps{v}", bufs=1, space="PSUM") as wps:
                for dy in range(KK):
                    pw = wps.tile([S * KK, C], F32R, tag="pw", name="pw")
                    nc.tensor.transpose(
                        pw[:], hB[:, dy * 80:dy * 80 + 80], identr[0:C, 0:C]
                    )
                    nc.vector.tensor_copy(hT5v[v][:, dy, :], pw[:])

        RB = 32
        NB = H // RB

        def s_init():
            with (
                tc.tile_pool(name="si_in", bufs=2) as pin,
                tc.tile_pool(name="si_out", bufs=3) as pout,
                tc.tile_pool(name="si_ps", bufs=2, space="PSUM") as pps,
            ):
                for b in range(NB):
                    xb = pin.tile([C, RB, W], F32R, tag="xb", name="xb")
                    nc.sync.dma_start(xb[:], x_in[:, RB * b:RB * b + RB, :])
                    nc.sync.dma_start(x_pad[:, RB * b + 2:RB * b + RB + 2, 2:258], xb[:])
                    for blk in range(4):
                        ps = pps.tile([S, 4, 512], F32, tag="ps", name="ps")
                        for j in range(4):
                            rp = blk * 4 + j
                            nc.tensor.matmul(
                                ps[:, j, :], wlt[:], xb[:, 2 * rp:2 * rp + 2, :],
                                start=True, stop=True,
                            )
                        ot = pout.tile([S, 4, 512], F32R, tag="ot", name="ot")
                        nc.scalar.activation(
                            ot[:], ps[:], mybir.ActivationFunctionType.Identity, bias=blt[:]
                        )
                        r0 = RB * b + 8 * blk + 2
                        nc.sync.dma_start(s_a[:, r0:r0 + 8, 2:258], ot[:])

        def conv_64_to_16(src_pad, dst, v, toggle=[0]):
            """dst[i,y,x] = sum_{c,dy,dx} h[c,i,4-dy,4-dx] src[c,y+dy-2,x+dx-2] + EPS"""
            h2s = h2v[v][:].rearrange("c (i f) -> c i f", i=S, f=25)
            with (
                tc.tile_pool(name="cn_in", bufs=2) as pin,
                tc.tile_pool(name="cn_out", bufs=3) as pout,
                tc.tile_pool(name="cn_ps", bufs=2, space="PSUM") as pps,
            ):
                for b in range(NB):
                    xb = pin.tile([128, RB + 4, PADW], F32R, tag="xb2", name="xb2")
                    nc.sync.dma_start(xb[0:C, :, :], src_pad[:, RB * b:RB * b + RB + 4, :])
                    nc.sync.dma_start(
                        xb[C:128, :, 0:259], src_pad[:, RB * b:RB * b + RB + 4, 1:260]
                    )
                    for blk in range(4):
                        ps = pps.tile([S, 4, 512], F32, tag="ps2", name="ps2")
                        for dy in range(KK):
                            for ka, dxa in ((0, 0), (1, 2), (2, 4)):
                                kp = 64 if dxa == 4 else 128
                                f = (4 - dy) * KK + (4 - dxa)
                                lhsT = h2s[0:kp, :, f]
                                for j in range(4):
                                    r = 2 * (blk * 4 + j) + dy
                                    rhs = xb[0:kp, r:r + 2, dxa:dxa + 256]
                                    nc.tensor.matmul(
                                        ps[:, j, :], lhsT, rhs,
                                        start=(dy == 0 and ka == 0),
                                        stop=(dy == KK - 1 and ka == 2),
                                    )
                        ot = pout.tile([S, 4, 512], F32, tag="ot2", name="ot2")
                        if toggle[0] % 2 == 0:
                            nc.scalar.activation(
                                ot[:], ps[:], mybir.ActivationFunctionType.Identity,
                                bias=epst[0:S],
                            )
                        else:
                            nc.vector.tensor_scalar_add(ot[:], ps[:], EPS)
                        toggle[0] += 1
                        y0 = RB * b + 8 * blk
                        nc.sync.dma_start(dst[:, y0:y0 + 8, :], ot[:])

        def conv_16_to_64(src_pad, v, dst_pad=None, dst_flat=None):
            """dst[c,y,x] = sum_{i,dy,dx} h[c,i,dy,dx] src[i,y+dy-2,x+dx-2]"""
            hT5 = hT5v[v]
            with (
                tc.tile_pool(name="ch_in", bufs=2) as pin,
                tc.tile_pool(name="ch_out", bufs=3) as pout,
                tc.tile_pool(name="ch_ps", bufs=2, space="PSUM") as pps,
            ):
                for b in range(NB):
                    sb5 = pin.tile([S * KK, RB + 4, W], F32R, tag="sb5", name="sb5")
                    for g in range(KK):
                        nc.sync.dma_start(
                            sb5[g * S:(g + 1) * S, :, :],
                            src_pad[:, RB * b:RB * b + RB + 4, g:g + W],
                        )
                    for blk in range(4):
                        ps = pps.tile([C, 4, 512], F32, tag="psh", name="psh")
                        for dy in range(KK):
                            for j in range(4):
                                r = 2 * (blk * 4 + j) + dy
                                nc.tensor.matmul(
                                    ps[:, j, :], hT5[:, dy, :], sb5[:, r:r + 2, :],
                                    start=(dy == 0), stop=(dy == KK - 1),
                                )
                        y0 = RB * b + 8 * blk
                        if dst_pad is not None:
                            ot = pout.tile([C, 4, 512], F32R, tag="oth", name="oth")
                            nc.vector.tensor_copy(ot[:], ps[:])
                            nc.sync.dma_start(dst_pad[:, y0 + 2:y0 + 10, 2:258], ot[:])
                        else:
                            ot = pout.tile([C, 4, 512], F32, tag="otf", name="otf")
                            nc.scalar.copy(ot[:], ps[:])
                            nc.sync.dma_start(dst_flat[:, y0:y0 + 8, :], ot[:])

        def s_update(s_src, s_dst):
            with tc.tile_pool(name="su", bufs=2) as pin:
                for b in range(4):
                    y0 = 64 * b
                    nb = pin.tile([128, 8, W], F32, tag="nb", name="nb")
                    db = pin.tile([128, 8, W], F32, tag="db", name="db")
                    sb_ = pin.tile([128, 8, W], F32R, tag="sb_", name="sb_")
                    for sg in range(8):
                        yy = y0 + 8 * sg
                        nc.sync.dma_start(nb[sg * S:(sg + 1) * S, :, :], num_t[:, yy:yy + 8, :])
                        nc.sync.dma_start(db[sg * S:(sg + 1) * S, :, :], den_t[:, yy:yy + 8, :])
                        nc.sync.dma_start(
                            sb_[sg * S:(sg + 1) * S, :, :], s_src[:, yy + 2:yy + 10, 2:258]
                        )
                    rec = pin.tile([128, 8, W], F32, tag="rec", name="rec")
                    nc.vector.reciprocal(rec[:], db[:])
                    rat = pin.tile([128, 8, W], F32, tag="rat", name="rat")
                    nc.vector.tensor_mul(rat[:], nb[:], rec[:])
                    so = pin.tile([128, 8, W], F32R, tag="so", name="so")
                    nc.vector.tensor_mul(so[:], sb_[:], rat[:])
                    for sg in range(8):
                        yy = y0 + 8 * sg
                        nc.sync.dma_start(
                            s_dst[:, yy + 2:yy + 10, 2:258], so[sg * S:(sg + 1) * S, :, :]
                        )

        def sconv_and_h_update(s_cur, v_old, v_new):
            RS = 16
            with (
                tc.tile_pool(name="sc_xx", bufs=2) as pxx,
                tc.tile_pool(name="sc_s5", bufs=2) as ps5,
                tc.tile_pool(name="sc_srow", bufs=2) as psr,
                tc.tile_pool(name="sc_xt", bufs=4) as pxt,
                tc.tile_pool(name="sc_ps", bufs=2, space="PSUM") as pps,
                tc.tile_pool(name="sc_acc", bufs=1, space="PSUM") as pacc,
                tc.tile_pool(name="sc_fin", bufs=1) as pfin,
            ):
                acc = pacc.tile([128, 400], F32, tag="acc", name="acc")
                nmm = 0
                total_mm = (H // RS) * RS * 2
                for b in range(H // RS):
                    y0 = RS * b + 2
                    xxb = pxx.tile([128, RS, W], F32R, tag="xxb", name="xxb")
                    nc.sync.dma_start(xxb[0:C, :, :], x_pad[:, y0:y0 + RS, 2:258])
                    nc.sync.dma_start(xxb[C:128, :, :], xh_b[:, y0:y0 + RS, 2:258])
                    s5b = ps5.tile([S * KK, RS + 4, W], F32R, tag="s5b", name="s5b")
                    for g in range(KK):
                        nc.sync.dma_start(
                            s5b[g * S:(g + 1) * S, :, :],
                            s_cur[:, y0 - 2:y0 + RS + 2, g:g + W],
                        )
                    srow = psr.tile([128, 2, RS + 4, S * KK], F32R, tag="srow", name="srow")
                    for yy in range(RS + 4):
                        for cb in range(2):
                            pt = pps.tile([128, S * KK], F32R, tag="pt", name="pt")
                            nc.tensor.transpose(
                                pt[:], s5b[:, yy, cb * 128:cb * 128 + 128],
                                identr[0:S * KK, 0:S * KK],
                            )
                            if (yy + cb) % 2 == 0:
                                nc.vector.tensor_copy(srow[:, cb, yy, :], pt[:])
                            else:
                                nc.scalar.copy(srow[:, cb, yy, :], pt[:])
                    for yy in range(RS):
                        for cb in range(2):
                            ptx = pps.tile([128, 128], F32R, tag="ptx", name="ptx")
                            nc.tensor.transpose(
                                ptx[:], xxb[:, yy, cb * 128:cb * 128 + 128], identr[:]
                            )
                            xt = pxt.tile([128, 128], F32R, tag="xt", name="xt")
                            if (yy + cb) % 2 == 0:
                                nc.scalar.copy(xt[:], ptx[:])
                            else:
                                nc.vector.tensor_copy(xt[:], ptx[:])
                            nc.tensor.matmul(
                                acc[:], xt[:], srow[:, cb, yy:yy + KK, :],
                                start=(nmm == 0), stop=(nmm == total_mm - 1),
                                skip_group_check=True,
                            )
                            nmm += 1
                # ---- h update ----
                a_t = pfin.tile([C, 400], F32, tag="a_t", name="a_t")
                nc.scalar.activation(
                    a_t[:], acc[0:C, :], mybir.ActivationFunctionType.Identity, bias=epst[0:C]
                )
                bhi = pfin.tile([128, 400], F32, tag="bhi", name="bhi")
                nc.scalar.activation(
                    bhi[C:128, :], acc[C:128, :], mybir.ActivationFunctionType.Identity,
                    bias=epst[C:128],
                )
                blo = pfin.tile([C, 400], F32, tag="blo", name="blo")
                nc.sync.dma_start(blo[:], bhi[C:128, :])
                rec = pfin.tile([C, 400], F32, tag="recb", name="recb")
                nc.vector.reciprocal(rec[:], blo[:])
                rr = pfin.tile([C, 400], F32, tag="rr", name="rr")
                nc.vector.tensor_mul(rr[:], a_t[:], rec[:])
                ho = hv[v_old][:].rearrange("c (i dy g) -> c i dy g", i=S, dy=KK, g=KK)
                hn = hv[v_new][:].rearrange("c (i dy g) -> c i dy g", i=S, dy=KK, g=KK)
                rrv = (
                    rr[:]
                    .rearrange("c (dy g i) -> c dy g i", dy=KK, g=KK, i=S)
                    .rearrange("c dy g i -> c i dy g")
                )
                nc.vector.tensor_mul(hn[:], ho[:], rrv[:])

        # ---- program ----
        build_weights(0)
        s_init()
        cur, nxt = s_a, s_b
        for it in range(2):
            conv_64_to_16(x_pad, num_t, it)
            conv_16_to_64(cur, it, dst_pad=xh_a)
            conv_64_to_16(xh_a, den_t, it)
            s_update(cur, nxt)
            conv_16_to_64(nxt, it, dst_pad=xh_b)
            sconv_and_h_update(nxt, it, it + 1)
            build_weights(it + 1)
            cur, nxt = nxt, cur
        conv_16_to_64(cur, 2, dst_flat=out_d)


def _build_nc():
    nc = bacc.Bacc("TRN2", target_bir_lowering=False)
    x_in = nc.dram_tensor("x", [C, H, W], F32R, kind="ExternalInput")
    h0_in = nc.dram_tensor("h0", [C, S * KK * KK], F32R, kind="ExternalInput")
    wlt_in = nc.dram_tensor("W_linT", [C, S], F32R, kind="ExternalInput")
    bl_in = nc.dram_tensor("b_lin", [S, 1], F32, kind="ExternalInput")
    id_in = nc.dram_tensor("ident", [128, 128], F32R, kind="ExternalInput")
    zr_in = nc.dram_tensor("zeros", [128, 520], F32R, kind="ExternalInput")
    out_d = nc.dram_tensor("out", [C, H, W], F32, kind="ExternalOutput")
    with tile.TileContext(nc) as tc:
        _emit(nc, tc, x_in, h0_in, wlt_in, bl_in, id_in, zr_in, out_d)
    nc.compile()
    return nc


def kernel(x, h0, W_lin, b_lin):
    from concourse.bass_utils import run_bass_kernel_spmd

    if "nc" not in _CACHE:
        _CACHE["nc"] = _build_nc()
    nc = _CACHE["nc"]

    x = np.ascontiguousarray(x, dtype=np.float32)
    h0f = np.ascontiguousarray(h0.reshape(C, S * KK * KK), dtype=np.float32)
    wlt = np.ascontiguousarray(W_lin.T, dtype=np.float32)
    blf = np.ascontiguousarray(np.asarray(b_lin).reshape(S, 1), dtype=np.float32)
    ident = np.eye(128, dtype=np.float32)

    in_maps = [
        {"x": np.ascontiguousarray(x[b]), "h0": h0f, "W_linT": wlt, "b_lin": blf,
         "ident": ident, "zeros": np.zeros((128, 520), dtype=np.float32)}
        for b in range(B)
    ]
    res = run_bass_kernel_spmd(nc, in_maps, core_ids=list(range(N_CORES)))
    _CACHE["last_result"] = res
    out = np.stack([res.results[b]["out"] for b in range(B)], axis=0)
    return out

